# revision 3
# baseline (speedup 1.0000x reference)
"""Cross-attention kernel for 8 Trainium2 NeuronCores.

Problem (hardcoded): x [4,4096,512], context [4,1024,768], 8 heads x 64,
inner 512. out = softmax((x@Wq)(ctx@Wk)^T / 8) @ (ctx@Wv) @ Wo + bo.

Sharding: 8 cores = 4 batches x 2 head-groups (4 heads each).
Core c handles batch b=c//2, heads [4g, 4g+4) with g=c%2:
  - Wq/Wk/Wv column-sliced, Wo row-sliced (tensor parallel over heads)
  - each core emits a partial [4096, 512]; host sums the two head-group
    partials per batch and adds bo.

v2 design: the kernel is ACT(exp)-bound — 128 exp calls of [128,1024]
at ~1.15us each = ~147us floor. Everything else must hide in ACT's
shadow. A flat software-pipelined stage loop over (qt, p, kc) keeps
the exp stream gapless:
  stage t: scores(t) [PE] -> exp(t) [ACT] -> AV(t-1) [PE] -> aux [PE]
PSUM: scores rotation 2x2 banks (never borrowed), AV accumulators
2x1 banks, aux accumulators (kproj/vproj/qproj/outproj) 2x1 banks.
Aux matmuls are slotted into per-stage PE slack via a schedule table.
"""

import os
import sys

for _p in ("/opt/trn_rl_repo", "/root/.axon_site/_ro/trn_rl_repo"):
    if os.path.isdir(_p) and _p not in sys.path:
        sys.path.append(_p)

import ml_dtypes
import numpy as np

BF16_NP = np.float16

import concourse.bass as bass  # noqa: E402
import concourse.mybir as mybir  # noqa: E402
import concourse.tile as tile  # noqa: E402
from concourse import bacc  # noqa: E402
from concourse import bass_utils  # noqa: E402

P = 128
B = 4
NQ = 4096  # queries per batch
DX = 512  # x feature dim (4 chunks of 128)
NC = 1024  # context length (8 key chunks of 128)
DC = 768  # context feature dim (6 chunks of 128)
DH = 64  # head dim
HPC = 4  # heads per core
COLS = HPC * DH  # 256 = per-core slice of the inner dim
DOUT = 512  # output dim

DXC = DX // P  # 4
DCC = DC // P  # 6
KC = NC // P  # 8
NQT = NQ // 512  # 8 query tiles of 512

F32 = mybir.dt.float32
BF16 = mybir.dt.float16
EXP = mybir.ActivationFunctionType.Exp
SCALE = DH**-0.5  # 0.125, folded into the exp activation's scale


def _emit(tc, nc, xT, ctxT, wq, wk, wv, wo, ones, out):
    with (
        tc.tile_pool(name="consts", bufs=1) as consts,
        tc.tile_pool(name="xstream", bufs=3) as xstream,
        tc.tile_pool(name="etile", bufs=6) as etile,
        tc.tile_pool(name="norm", bufs=2) as norm,
        tc.tile_pool(name="dscr", bufs=4, space="DRAM") as dscr,
        tc.tile_pool(name="ps_scores", bufs=2, space="PSUM") as ps_scores,
        tc.tile_pool(name="ps_av", bufs=2, space="PSUM") as ps_av,
        tc.tile_pool(name="ps_aux", bufs=2, space="PSUM") as ps_aux,
    ):
        # ---- weights + context into SBUF (feature dim on partitions) ----
        wq_sb = consts.tile([P, DXC, COLS], BF16, tag="wq", name="wq_sb")
        wk_sb = consts.tile([P, DCC, COLS], BF16, tag="wk", name="wk_sb")
        wv_sb = consts.tile([P, DCC, COLS], BF16, tag="wv", name="wv_sb")
        wo_sb = consts.tile([P, 2, DOUT], BF16, tag="wo", name="wo_sb")
        ctx_pool_cm = tc.tile_pool(name="ctxpool", bufs=1)
        ctx_pool = ctx_pool_cm.__enter__()
        ctxT_sb = ctx_pool.tile([P, DCC, NC], BF16, tag="ctxT", name="ctxT_sb")
        nc.sync.dma_start(wk_sb[:], wk.rearrange("(c p) n -> p c n", p=P))
        nc.sync.dma_start(ctxT_sb[:], ctxT.rearrange("(c p) n -> p c n", p=P))
        nc.sync.dma_start(wq_sb[:], wq.rearrange("(c p) n -> p c n", p=P))
        nc.sync.dma_start(wv_sb[:], wv.rearrange("(c p) n -> p c n", p=P))
        nc.sync.dma_start(wo_sb[:], wo.rearrange("(c p) n -> p c n", p=P))

        kT_sb = [consts.tile([P, NC], BF16, tag=f"kT{p}", name=f"kT{p}") for p in range(2)]
        # v_sb[:, kc, h, 0:64] = V for head h, key chunk kc; [..., 64] = 1.0
        v_sb = consts.tile([P, KC, HPC, DH + 1], BF16, tag="v", name="v_sb")
        nc.sync.dma_start(
            v_sb[:, :, :, DH : DH + 1].rearrange("p a b o -> p (a b o)"),
            ones.to_broadcast((P, KC * HPC)),
        )

        # ---- aux emitters (each borrows a short-lived ps_aux tile) ----
        def kproj(p, ks):
            acc = ps_aux.tile([P, DOUT], F32, tag="aux", name="kproj_acc")
            for ch in range(DCC):
                nc.tensor.matmul(
                    acc[:],
                    wk_sb[:, ch, p * P : (p + 1) * P],
                    ctxT_sb[:, ch, ks * 512 : (ks + 1) * 512],
                    start=(ch == 0),
                    stop=(ch == DCC - 1),
                )
            nc.vector.tensor_copy(kT_sb[p][:, ks * 512 : (ks + 1) * 512], acc[:])

        def vproj(kc):
            acc = ps_aux.tile([P, DOUT], F32, tag="aux", name="vproj_acc")
            for ch in range(DCC):
                nc.tensor.matmul(
                    acc[:, 0:COLS],
                    ctxT_sb[:, ch, kc * P : (kc + 1) * P],
                    wv_sb[:, ch, :],
                    start=(ch == 0),
                    stop=(ch == DCC - 1),
                )
            nc.vector.tensor_copy(
                v_sb[:, kc, :, 0:DH], acc[:, 0:COLS].rearrange("p (h d) -> p h d", d=DH)
            )

        xt_sb = {}

        def xt_load(qs):
            xt = xstream.tile([P, DXC, 512], BF16, tag="xt", name="xt")
            xt_sb[qs] = xt
            nc.sync.dma_start(
                xt[:],
                xT.rearrange("(c p) q -> p c q", p=P)[:, :, qs * 512 : (qs + 1) * 512],
            )

        qT_sb = {}
        _qp_state = {}

        def qproj_half(qs, p, half):
            # half 0: chunks 0-1 (allocates acc); half 1: chunks 2-3 + copy out
            if half == 0:
                acc = ps_aux.tile([P, DOUT], F32, tag="aux", name="qproj_acc")
                _qp_state[(p, qs)] = acc
            acc = _qp_state[(p, qs)]
            for ch in (0, 1) if half == 0 else (2, 3):
                nc.tensor.matmul(
                    acc[:],
                    wq_sb[:, ch, p * P : (p + 1) * P],
                    xt_sb[qs][:, ch, :],
                    start=(ch == 0),
                    stop=(ch == DXC - 1),
                )
            if half == 1:
                qt_t = consts.tile([P, 512], BF16, tag=f"qT{p}_{qs}", name=f"qT{p}_{qs}")
                qT_sb[(p, qs)] = qt_t
                nc.vector.tensor_copy(qt_t[:], acc[:])
                del _qp_state[(p, qs)]

        attnT_all = {}

        def outproj_sub(qt, sub):
            o = ps_aux.tile([P, DOUT], F32, tag="aux", name="oproj_acc")
            for p in range(2):
                nc.tensor.matmul(
                    o[:],
                    attnT_all[(p, qt)][:, sub * P : (sub + 1) * P],
                    wo_sb[:, p, :],
                    start=(p == 0),
                    stop=(p == 1),
                )
            ostage = norm.tile([P, DOUT], F32, tag="ostage", name="ostage_t")
            nc.vector.tensor_copy(ostage[:], o[:])
            row = qt * 512 + sub * P
            nc.sync.dma_start(out[row : row + P, :], ostage[:])

        def attn_normalize(qt, p, accs):
            at_t = consts.tile([P, 512], BF16, tag=f"attnT{p}_{qt}", name=f"attnT{p}_{qt}")
            attnT_all[(p, qt)] = at_t
            dstage = norm.tile([DH + 1, 2, 512], F32, tag="denom", name="den_t")
            for j in range(2):
                nc.vector.tensor_copy(
                    dstage[DH : DH + 1, j, :], accs[j][DH : DH + 1, :]
                )
            dden = dscr.tile([1, 1024], F32, tag="dden", name="dden_t")
            nc.gpsimd.dma_start(dden[:], dstage[DH : DH + 1, :, :])
            rt = norm.tile([P, 8], F32, tag="rt", name="rt_t")
            nc.gpsimd.dma_start(rt[:], dden[0, :].rearrange("(p f) -> p f", p=P))
            nc.vector.reciprocal(rt[:], rt[:])
            drec = dscr.tile([1, 1024], F32, tag="drec", name="drec_t")
            nc.gpsimd.dma_start(drec[:], rt[:])
            for j in range(2):
                rec = norm.tile([DH, 512], F32, tag="recip", name="recip_t")
                nc.gpsimd.dma_start(
                    rec[:],
                    drec[:, j * 512 : (j + 1) * 512].to_broadcast((DH, 512)),
                )
                if j == 0:
                    nc.vector.tensor_mul(at_t[0:DH, :], accs[j][0:DH, :], rec[:])
                else:
                    tmp = norm.tile([DH, 512], BF16, tag="normtmp", name="normtmp_t")
                    nc.vector.tensor_mul(tmp[:], accs[j][0:DH, :], rec[:])
                    # engines cannot shift partitions; DMA moves the odd
                    # head's rows into partitions 64-127
                    nc.gpsimd.dma_start(at_t[DH:P, :], tmp[:])

        # ---- aux schedule: (qt, s) -> list of thunks; s = p*KC + kc ----
        aux = {}

        def at(qt, s, fn):
            aux.setdefault((qt, s), []).append(fn)

        # qt0: remaining kproj halves + vproj kc2..kc7 + qproj(qt0,p1)
        at(0, 0, lambda: kproj(0, 1))
        at(0, 0, lambda: vproj(2))
        at(0, 1, lambda: kproj(1, 0))
        at(0, 1, lambda: vproj(3))
        at(0, 2, lambda: kproj(1, 1))
        at(0, 2, lambda: vproj(4))
        at(0, 3, lambda: vproj(5))
        at(0, 3, lambda: qproj_half(0, 1, 0))
        at(0, 4, lambda: vproj(6))
        at(0, 4, lambda: qproj_half(0, 1, 1))
        at(0, 5, lambda: vproj(7))
        # qt0 also needs qproj(qt1) late
        at(0, 8, lambda: xt_load(1))
        at(0, 10, lambda: qproj_half(1, 0, 0))
        at(0, 11, lambda: qproj_half(1, 0, 1))
        at(0, 12, lambda: qproj_half(1, 1, 0))
        at(0, 13, lambda: qproj_half(1, 1, 1))
        # steady qts: outproj(qt-1) at s4..s7 (after normalize(qt-1,p1)'s
        # ~4us DMA chain lands — earlier would block the in-order PE queue),
        # qproj(qt+1) at s8..s11
        for qt in range(1, NQT):
            for sub in range(4):
                at(qt, 4 + sub, lambda qt=qt, sub=sub: outproj_sub(qt - 1, sub))
            if qt + 1 < NQT:
                at(qt, 0, lambda qt=qt: xt_load(qt + 1))
                at(qt, 8, lambda qt=qt: qproj_half(qt + 1, 0, 0))
                at(qt, 9, lambda qt=qt: qproj_half(qt + 1, 0, 1))
                at(qt, 10, lambda qt=qt: qproj_half(qt + 1, 1, 0))
                at(qt, 11, lambda qt=qt: qproj_half(qt + 1, 1, 1))

        # ---- prologue: minimal chain to first scores ----
        kproj(0, 0)
        xt_load(0)
        qproj_half(0, 0, 0)
        qproj_half(0, 0, 1)
        vproj(0)
        vproj(1)
        ctx_release_stage = 6  # ctxT freed once kproj/vproj all emitted

        # ---- flat pipelined stage loop over (qt, p, kc) ----
        stages = [(qt, p, kc) for qt in range(NQT) for p in range(2) for kc in range(KC)]
        av_accs = {}
        prev = None  # (qt, p, kc, ex_tile)

        def emit_av(qt, p, kc, ex):
            if kc == 0:
                av_accs[(qt, p)] = [
                    ps_av.tile([DH + 1, 512], F32, tag="av", name="av_acc")
                    for _ in range(2)
                ]
            accs = av_accs[(qt, p)]
            for j in range(2):
                nc.tensor.matmul(
                    accs[j][:],
                    v_sb[:, kc, 2 * p + j, :],
                    ex[:, j, :],
                    start=(kc == 0),
                    stop=(kc == KC - 1),
                )
            if kc == KC - 1:
                attn_normalize(qt, p, accs)
                del av_accs[(qt, p)]

        released_ctx = False
        for t, (qt, p, kc) in enumerate(stages):
            s = p * KC + kc
            # scores for stage t
            sc = ps_scores.tile([P, 2, 512], F32, tag="scores", name="scores_ps")
            qt_t = qT_sb[(p, qt)]
            for j in range(2):
                nc.tensor.matmul(
                    sc[:, j, :],
                    kT_sb[p][j * DH : (j + 1) * DH, kc * P : (kc + 1) * P],
                    qt_t[j * DH : (j + 1) * DH, :],
                    start=True,
                    stop=True,
                )
            # exp for stage t
            ex = etile.tile([P, 2, 512], BF16, tag="exp", name="exp_sb")
            nc.scalar.activation(ex[:], sc[:], EXP, scale=SCALE)
            # AV for stage t-1
            if prev is not None:
                emit_av(*prev)
            prev = (qt, p, kc, ex)
            # aux work for this stage
            for fn in aux.get((qt, s), ()):
                fn()
            if qt == 0 and s == ctx_release_stage and not released_ctx:
                released_ctx = True
                ctx_pool_cm.__exit__(None, None, None)
        emit_av(*prev)
        for sub in range(4):
            outproj_sub(NQT - 1, sub)


def _build():
    nc = bacc.Bacc(
        "TRN2", target_bir_lowering=False, debug=False, enable_asserts=False
    )
    xT = nc.dram_tensor("xT", [DX, NQ], BF16, kind="ExternalInput").ap()
    ctxT = nc.dram_tensor("ctxT", [DC, NC], BF16, kind="ExternalInput").ap()
    wq = nc.dram_tensor("wq", [DX, COLS], BF16, kind="ExternalInput").ap()
    wk = nc.dram_tensor("wk", [DC, COLS], BF16, kind="ExternalInput").ap()
    wv = nc.dram_tensor("wv", [DC, COLS], BF16, kind="ExternalInput").ap()
    wo = nc.dram_tensor("wo", [COLS, DOUT], BF16, kind="ExternalInput").ap()
    ones = nc.dram_tensor("ones", [1, KC * HPC], BF16, kind="ExternalInput").ap()
    out = nc.dram_tensor("out", [NQ, DOUT], F32, kind="ExternalOutput").ap()
    with tile.TileContext(nc) as tc:
        _emit(tc, nc, xT, ctxT, wq, wk, wv, wo, ones, out)
    nc.compile()
    return nc


_NC = None


def _get_nc():
    global _NC
    if _NC is None:
        _NC = _build()
    return _NC


def _in_maps(x, context, Wq, Wk, Wv, Wo):
    maps = []
    for c in range(8):
        b, g = c // 2, c % 2
        cs = slice(g * COLS, (g + 1) * COLS)
        maps.append(
            {
                "xT": np.ascontiguousarray(x[b].T.astype(BF16_NP)),
                "ctxT": np.ascontiguousarray(context[b].T.astype(BF16_NP)),
                "wq": np.ascontiguousarray(Wq[:, cs].astype(BF16_NP)),
                "wk": np.ascontiguousarray(Wk[:, cs].astype(BF16_NP)),
                "wv": np.ascontiguousarray(Wv[:, cs].astype(BF16_NP)),
                "wo": np.ascontiguousarray(Wo[cs, :].astype(BF16_NP)),
                "ones": np.ones((1, KC * HPC), BF16_NP),
            }
        )
    return maps


def _execute(in_maps, **kw):
    return bass_utils.run_bass_kernel_spmd(
        _get_nc(), in_maps, core_ids=list(range(8)), **kw
    )


def kernel(x, context, Wq, Wk, Wv, Wo, bo):
    x = np.asarray(x, np.float32)
    context = np.asarray(context, np.float32)
    Wq = np.asarray(Wq, np.float32)
    Wk = np.asarray(Wk, np.float32)
    Wv = np.asarray(Wv, np.float32)
    Wo = np.asarray(Wo, np.float32)
    bo = np.asarray(bo, np.float32)
    res = _execute(_in_maps(x, context, Wq, Wk, Wv, Wo))
    parts = [r["out"] for r in res.results]
    out = np.empty((B, NQ, DOUT), np.float32)
    for b in range(B):
        out[b] = parts[2 * b] + parts[2 * b + 1] + bo[None, :]
    return out


# revision 8
# speedup vs baseline: 1.8934x; 1.8934x over previous
"""Cross-attention kernel for 8 Trainium2 NeuronCores.

Problem (hardcoded): x [4,4096,512], context [4,1024,768], 8 heads x 64,
inner 512. out = softmax((x@Wq)(ctx@Wk)^T / 8) @ (ctx@Wv) @ Wo + bo.

Sharding: 8 cores = 4 batches x 2 head-groups (4 heads each).
Core c handles batch b=c//2, heads [4g, 4g+4) with g=c%2:
  - Wq/Wk/Wv column-sliced, Wo row-sliced (tensor parallel over heads)
  - each core emits a partial [4096, 512]; host sums the two head-group
    partials per batch and adds bo.

v2 design: the kernel is ACT(exp)-bound — 128 exp calls of [128,1024]
at ~1.15us each = ~147us floor. Everything else must hide in ACT's
shadow. A flat software-pipelined stage loop over (qt, p, kc) keeps
the exp stream gapless:
  stage t: scores(t) [PE] -> exp(t) [ACT] -> AV(t-1) [PE] -> aux [PE]
PSUM: scores rotation 2x2 banks (never borrowed), AV accumulators
2x1 banks, aux accumulators (kproj/vproj/qproj/outproj) 2x1 banks.
Aux matmuls are slotted into per-stage PE slack via a schedule table.
"""

import os
import sys

for _p in ("/opt/trn_rl_repo", "/root/.axon_site/_ro/trn_rl_repo"):
    if os.path.isdir(_p) and _p not in sys.path:
        sys.path.append(_p)

import ml_dtypes
import numpy as np

BF16_NP = np.float16

import concourse.bass as bass  # noqa: E402
import concourse.mybir as mybir  # noqa: E402
import concourse.tile as tile  # noqa: E402
from concourse import bacc  # noqa: E402
from concourse import bass_utils  # noqa: E402

P = 128
B = 4
NQ = 4096  # queries per batch
DX = 512  # x feature dim (4 chunks of 128)
NC = 1024  # context length (8 key chunks of 128)
DC = 768  # context feature dim (6 chunks of 128)
DH = 64  # head dim
HPC = 4  # heads per core
COLS = HPC * DH  # 256 = per-core slice of the inner dim
DOUT = 512  # output dim

DXC = DX // P  # 4
DCC = DC // P  # 6
KC = NC // P  # 8
NQT = NQ // 512  # 8 query tiles of 512

F32 = mybir.dt.float32
BF16 = mybir.dt.float16
EXP = mybir.ActivationFunctionType.Exp
SCALE = DH**-0.5  # 0.125, folded into the exp activation's scale


def _emit(tc, nc, xT, ctxT, wq, wk, wv, wo, out):
    with (
        tc.tile_pool(name="consts", bufs=1) as consts,
        tc.tile_pool(name="xstream", bufs=3) as xstream,
        tc.tile_pool(name="etile", bufs=6) as etile,
        tc.tile_pool(name="norm", bufs=2) as norm,
        tc.tile_pool(name="dscr", bufs=4, space="DRAM") as dscr,
        tc.tile_pool(name="ps_scores", bufs=2, space="PSUM") as ps_scores,
        tc.tile_pool(name="ps_av", bufs=2, space="PSUM") as ps_av,
        tc.tile_pool(name="ps_aux", bufs=2, space="PSUM") as ps_aux,
    ):
        # ---- weights + context into SBUF (feature dim on partitions) ----
        wq_sb = consts.tile([P, DXC, COLS], BF16, tag="wq", name="wq_sb")
        wk_sb = consts.tile([P, DCC, COLS], BF16, tag="wk", name="wk_sb")
        wv_sb = consts.tile([P, DCC, COLS], BF16, tag="wv", name="wv_sb")
        wo_sb = consts.tile([P, 2, DOUT], BF16, tag="wo", name="wo_sb")
        ctx_pool_cm = tc.tile_pool(name="ctxpool", bufs=1)
        ctx_pool = ctx_pool_cm.__enter__()
        ctxT_sb = ctx_pool.tile([P, DCC, NC], BF16, tag="ctxT", name="ctxT_sb")
        nc.sync.dma_start(wk_sb[:], wk.rearrange("(c p) n -> p c n", p=P))
        nc.sync.dma_start(ctxT_sb[:], ctxT.rearrange("(c p) n -> p c n", p=P))
        nc.sync.dma_start(wq_sb[:], wq.rearrange("(c p) n -> p c n", p=P))
        nc.sync.dma_start(wv_sb[:], wv.rearrange("(c p) n -> p c n", p=P))
        nc.sync.dma_start(wo_sb[:], wo.rearrange("(c p) n -> p c n", p=P))

        kT_sb = [consts.tile([P, NC], BF16, tag=f"kT{p}", name=f"kT{p}") for p in range(2)]
        # v_sb[:, kc, h, 0:64] = V for head h, key chunk kc; [..., 64] = 1.0
        # (memset, NOT a broadcast DMA: a 2-byte-element strided DMA shatters
        # into 4096 tiny packets that clog every hw DMA queue for ~30us)
        v_sb = consts.tile([P, KC, HPC, DH + 1], BF16, tag="v", name="v_sb")
        nc.vector.memset(
            v_sb[:, :, :, DH : DH + 1].rearrange("p a b o -> p (a b o)"), 1.0
        )

        # ---- aux emitters (each borrows a short-lived ps_aux tile) ----
        def kproj(p, ks):
            acc = ps_aux.tile([P, DOUT], F32, tag="aux", name="kproj_acc")
            for ch in range(DCC):
                nc.tensor.matmul(
                    acc[:],
                    wk_sb[:, ch, p * P : (p + 1) * P],
                    ctxT_sb[:, ch, ks * 512 : (ks + 1) * 512],
                    start=(ch == 0),
                    stop=(ch == DCC - 1),
                )
            nc.vector.tensor_copy(kT_sb[p][:, ks * 512 : (ks + 1) * 512], acc[:])

        def vproj(kc):
            acc = ps_aux.tile([P, DOUT], F32, tag="aux", name="vproj_acc")
            for ch in range(DCC):
                nc.tensor.matmul(
                    acc[:, 0:COLS],
                    ctxT_sb[:, ch, kc * P : (kc + 1) * P],
                    wv_sb[:, ch, :],
                    start=(ch == 0),
                    stop=(ch == DCC - 1),
                )
            nc.vector.tensor_copy(
                v_sb[:, kc, :, 0:DH], acc[:, 0:COLS].rearrange("p (h d) -> p h d", d=DH)
            )

        xt_sb = {}

        def xt_load(qs):
            xt = xstream.tile([P, DXC, 512], BF16, tag="xt", name="xt")
            xt_sb[qs] = xt
            nc.sync.dma_start(
                xt[:],
                xT.rearrange("(c p) q -> p c q", p=P)[:, :, qs * 512 : (qs + 1) * 512],
            )

        qT_sb = {}
        _qp_state = {}

        def qproj_half(qs, p, half):
            # half 0: chunks 0-1 (allocates acc); half 1: chunks 2-3 + copy out
            if half == 0:
                acc = ps_aux.tile([P, DOUT], F32, tag="aux", name="qproj_acc")
                _qp_state[(p, qs)] = acc
            acc = _qp_state[(p, qs)]
            for ch in (0, 1) if half == 0 else (2, 3):
                nc.tensor.matmul(
                    acc[:],
                    wq_sb[:, ch, p * P : (p + 1) * P],
                    xt_sb[qs][:, ch, :],
                    start=(ch == 0),
                    stop=(ch == DXC - 1),
                )
            if half == 1:
                qt_t = consts.tile([P, 512], BF16, tag=f"qT{p}_{qs}", name=f"qT{p}_{qs}")
                qT_sb[(p, qs)] = qt_t
                nc.vector.tensor_copy(qt_t[:], acc[:])
                del _qp_state[(p, qs)]

        attnT_all = {}

        def outproj_sub(qt, sub):
            o = ps_aux.tile([P, DOUT], F32, tag="aux", name="oproj_acc")
            for p in range(2):
                nc.tensor.matmul(
                    o[:],
                    attnT_all[(p, qt)][:, sub * P : (sub + 1) * P],
                    wo_sb[:, p, :],
                    start=(p == 0),
                    stop=(p == 1),
                )
            ostage = norm.tile([P, DOUT], F32, tag="ostage", name="ostage_t")
            nc.vector.tensor_copy(ostage[:], o[:])
            row = qt * 512 + sub * P
            nc.sync.dma_start(out[row : row + P, :], ostage[:])

        def attn_normalize(qt, p, accs):
            at_t = consts.tile([P, 512], BF16, tag=f"attnT{p}_{qt}", name=f"attnT{p}_{qt}")
            attnT_all[(p, qt)] = at_t
            # evacuate the PSUM accumulators to SBUF immediately: the ps_av
            # ring frees as soon as these copies land, so the next p-loop's
            # AV matmuls never wait out the ~7us normalize DMA chain below
            dstage = norm.tile([DH + 1, 2, 512], F32, tag="denom", name="den_t")
            for j in range(2):
                nc.vector.tensor_copy(dstage[:, j, :], accs[j][:])
            dden = dscr.tile([1, 1024], F32, tag="dden", name="dden_t")
            nc.gpsimd.dma_start(dden[:], dstage[DH : DH + 1, :, :])
            rt = norm.tile([P, 8], F32, tag="rt", name="rt_t")
            nc.gpsimd.dma_start(rt[:], dden[0, :].rearrange("(p f) -> p f", p=P))
            nc.vector.reciprocal(rt[:], rt[:])
            drec = dscr.tile([1, 1024], F32, tag="drec", name="drec_t")
            nc.gpsimd.dma_start(drec[:], rt[:])
            for j in range(2):
                rec = norm.tile([DH, 512], F32, tag="recip", name="recip_t")
                nc.gpsimd.dma_start(
                    rec[:],
                    drec[:, j * 512 : (j + 1) * 512].to_broadcast((DH, 512)),
                )
                if j == 0:
                    nc.vector.tensor_mul(at_t[0:DH, :], dstage[0:DH, j, :], rec[:])
                else:
                    tmp = norm.tile([DH, 512], BF16, tag="normtmp", name="normtmp_t")
                    nc.vector.tensor_mul(tmp[:], dstage[0:DH, j, :], rec[:])
                    # engines cannot shift partitions; DMA moves the odd
                    # head's rows into partitions 64-127
                    nc.gpsimd.dma_start(at_t[DH:P, :], tmp[:])

        # ---- aux schedule: (qt, s) -> list of thunks; s = p*KC + kc ----
        aux = {}

        def at(qt, s, fn):
            aux.setdefault((qt, s), []).append(fn)

        # qt0: remaining kproj halves + vproj kc2..kc7 + qproj(qt0,p1)
        at(0, 0, lambda: kproj(0, 1))
        at(0, 0, lambda: vproj(2))
        at(0, 1, lambda: kproj(1, 0))
        at(0, 1, lambda: vproj(3))
        at(0, 2, lambda: kproj(1, 1))
        at(0, 2, lambda: vproj(4))
        at(0, 3, lambda: vproj(5))
        at(0, 3, lambda: qproj_half(0, 1, 0))
        at(0, 4, lambda: vproj(6))
        at(0, 4, lambda: qproj_half(0, 1, 1))
        at(0, 5, lambda: vproj(7))
        # qt0 also needs qproj(qt1) late
        at(0, 8, lambda: xt_load(1))
        at(0, 10, lambda: qproj_half(1, 0, 0))
        at(0, 11, lambda: qproj_half(1, 0, 1))
        at(0, 12, lambda: qproj_half(1, 1, 0))
        at(0, 13, lambda: qproj_half(1, 1, 1))
        # steady qts: qproj(qt+1) at s5..s8; outproj(qt-1) at s9..s12 (after
        # normalize(qt-1,p1)'s ~7us DMA chain lands around s7 — any earlier
        # would block the in-order PE queue and starve ACT)
        for qt in range(1, NQT):
            for sub in range(4):
                at(qt, 9 + sub, lambda qt=qt, sub=sub: outproj_sub(qt - 1, sub))
            if qt + 1 < NQT:
                at(qt, 0, lambda qt=qt: xt_load(qt + 1))
                at(qt, 5, lambda qt=qt: qproj_half(qt + 1, 0, 0))
                at(qt, 6, lambda qt=qt: qproj_half(qt + 1, 0, 1))
                at(qt, 7, lambda qt=qt: qproj_half(qt + 1, 1, 0))
                at(qt, 8, lambda qt=qt: qproj_half(qt + 1, 1, 1))

        # ---- prologue: minimal chain to first scores ----
        kproj(0, 0)
        xt_load(0)
        qproj_half(0, 0, 0)
        qproj_half(0, 0, 1)
        vproj(0)
        vproj(1)
        ctx_release_stage = 6  # ctxT freed once kproj/vproj all emitted

        # ---- flat pipelined stage loop over (qt, p, kc) ----
        stages = [(qt, p, kc) for qt in range(NQT) for p in range(2) for kc in range(KC)]
        av_accs = {}
        prev = None  # (qt, p, kc, ex_tile)

        def emit_av(qt, p, kc, ex):
            if kc == 0:
                av_accs[(qt, p)] = [
                    ps_av.tile([DH + 1, 512], F32, tag="av", name="av_acc")
                    for _ in range(2)
                ]
            accs = av_accs[(qt, p)]
            for j in range(2):
                nc.tensor.matmul(
                    accs[j][:],
                    v_sb[:, kc, 2 * p + j, :],
                    ex[:, j, :],
                    start=(kc == 0),
                    stop=(kc == KC - 1),
                )
            if kc == KC - 1:
                attn_normalize(qt, p, accs)
                del av_accs[(qt, p)]

        released_ctx = False
        for t, (qt, p, kc) in enumerate(stages):
            s = p * KC + kc
            # scores for stage t
            sc = ps_scores.tile([P, 2, 512], F32, tag="scores", name="scores_ps")
            qt_t = qT_sb[(p, qt)]
            for j in range(2):
                nc.tensor.matmul(
                    sc[:, j, :],
                    kT_sb[p][j * DH : (j + 1) * DH, kc * P : (kc + 1) * P],
                    qt_t[j * DH : (j + 1) * DH, :],
                    start=True,
                    stop=True,
                )
            # exp for stage t
            ex = etile.tile([P, 2, 512], BF16, tag="exp", name="exp_sb")
            nc.scalar.activation(ex[:], sc[:], EXP, scale=SCALE)
            # AV for stage t-1
            if prev is not None:
                emit_av(*prev)
            prev = (qt, p, kc, ex)
            # aux work for this stage
            for fn in aux.get((qt, s), ()):
                fn()
            if qt == 0 and s == ctx_release_stage and not released_ctx:
                released_ctx = True
                ctx_pool_cm.__exit__(None, None, None)
        emit_av(*prev)
        for sub in range(4):
            outproj_sub(NQT - 1, sub)


def _build():
    nc = bacc.Bacc(
        "TRN2", target_bir_lowering=False, debug=False, enable_asserts=False
    )
    xT = nc.dram_tensor("xT", [DX, NQ], BF16, kind="ExternalInput").ap()
    ctxT = nc.dram_tensor("ctxT", [DC, NC], BF16, kind="ExternalInput").ap()
    wq = nc.dram_tensor("wq", [DX, COLS], BF16, kind="ExternalInput").ap()
    wk = nc.dram_tensor("wk", [DC, COLS], BF16, kind="ExternalInput").ap()
    wv = nc.dram_tensor("wv", [DC, COLS], BF16, kind="ExternalInput").ap()
    wo = nc.dram_tensor("wo", [COLS, DOUT], BF16, kind="ExternalInput").ap()
    out = nc.dram_tensor("out", [NQ, DOUT], F32, kind="ExternalOutput").ap()
    with tile.TileContext(nc) as tc:
        _emit(tc, nc, xT, ctxT, wq, wk, wv, wo, out)
    nc.compile()
    return nc


_NC = None


def _get_nc():
    global _NC
    if _NC is None:
        _NC = _build()
    return _NC


def _in_maps(x, context, Wq, Wk, Wv, Wo):
    maps = []
    for c in range(8):
        b, g = c // 2, c % 2
        cs = slice(g * COLS, (g + 1) * COLS)
        maps.append(
            {
                "xT": np.ascontiguousarray(x[b].T.astype(BF16_NP)),
                "ctxT": np.ascontiguousarray(context[b].T.astype(BF16_NP)),
                "wq": np.ascontiguousarray(Wq[:, cs].astype(BF16_NP)),
                "wk": np.ascontiguousarray(Wk[:, cs].astype(BF16_NP)),
                "wv": np.ascontiguousarray(Wv[:, cs].astype(BF16_NP)),
                "wo": np.ascontiguousarray(Wo[cs, :].astype(BF16_NP)),
            }
        )
    return maps


def _execute(in_maps, **kw):
    return bass_utils.run_bass_kernel_spmd(
        _get_nc(), in_maps, core_ids=list(range(8)), **kw
    )


def kernel(x, context, Wq, Wk, Wv, Wo, bo):
    x = np.asarray(x, np.float32)
    context = np.asarray(context, np.float32)
    Wq = np.asarray(Wq, np.float32)
    Wk = np.asarray(Wk, np.float32)
    Wv = np.asarray(Wv, np.float32)
    Wo = np.asarray(Wo, np.float32)
    bo = np.asarray(bo, np.float32)
    res = _execute(_in_maps(x, context, Wq, Wk, Wv, Wo))
    parts = [r["out"] for r in res.results]
    out = np.empty((B, NQ, DOUT), np.float32)
    for b in range(B):
        out[b] = parts[2 * b] + parts[2 * b + 1] + bo[None, :]
    return out
